# revision 1
# baseline (speedup 1.0000x reference)
"""Holt-Winters exponential smoothing (level/trend/seasonal, P=7) on 8 Trainium2
NeuronCores.

Math: the per-row recurrence is linear in a 9-dim state
s = [level, trend, buf_0..buf_6]:  s_t = A_{t%7} s_{t-1} + c_{t%7} x_t.
Steps t=1..4095 are processed in 117 chunks of C=35 steps (35 % 7 == 0 so every
chunk sees the same slot pattern and shares one coefficient set), grouped into
9 groups of G=13 chunks.  Per chunk the outputs are one K=123 matmul
  Y_c (105,B) = lhsT.T @ [X_hi; X_lo; X_hi; sig_hi; sig_lo]
plus one K=117 matmul against the group state tile; the chunk-entry states
sigma_c come from a per-group prefix-scan matmul over the group's stacked
inputs (one K=105 matmul per chunk + 3 state matmuls per group).  The only
sequential dependency is the 9-link group chain.

Precision: inputs and coefficients are split hi/lo into bf16 pairs
(x = hi + lo, residual ~2^-18 relative).  Full precision needs the three
products Wh.Xh, Wh.Xl, Wl.Xh; the duplicated X_hi rows let all three run in a
single K-stacked bf16 matmul (1 cycle/row vs fp32's 4).  All products are
exact in fp32 PSUM accumulation; dropped Wl.Xl is ~2^-18 relative.

Sharding: pure data-parallel over the batch axis (1024 rows per core).
"""

import numpy as np

P = 7
C = 35            # chunk size (steps); 35 % 7 == 0
G = 13            # chunks per group
NG = 9            # groups; NG*G*C == L-1
L = 4096
B = 8192
NCORES = 8
BL = B // NCORES  # 1024 batch rows per core
NHALF = 512       # matmul moving-dim tile (fp32 PSUM bank limit)


def _sigmoid(z):
    return 1.0 / (1.0 + np.exp(-z))


def _step_mats(a, b, g):
    """A_i (9x9), c_i (9,) for seasonal slot i, float64."""
    A, c = [], []
    for i in range(P):
        col = 2 + i
        Ai = np.zeros((9, 9), np.float64)
        ci = np.zeros(9, np.float64)
        Ai[0, 0] = 1 - a
        Ai[0, 1] = 1 - a
        Ai[0, col] += -a
        Ai[1, 0] = -a * b
        Ai[1, 1] = 1 - a * b
        Ai[1, col] += -a * b
        for j in range(P):
            Ai[2 + j, 2 + j] = 1.0
        Ai[col, :] = 0.0
        Ai[col, 0] = -g * (1 - a)
        Ai[col, 1] = -g * (1 - a)
        Ai[col, col] = g * a + 1 - g
        ci[0] = a
        ci[1] = a * b
        ci[col] = g * (1 - a)
        A.append(Ai)
        c.append(ci)
    return A, c


def _hi_lo(x):
    import ml_dtypes
    hi = x.astype(np.float32).astype(ml_dtypes.bfloat16)
    lo = (x.astype(np.float32) - hi.astype(np.float32)).astype(ml_dtypes.bfloat16)
    return hi, lo


def _build_coeffs(alpha, beta, gamma):
    """Host-precomputed stationary matrices (float64 -> bf16 hi/lo splits).

    X-tile row layout (123 rows): [X_hi; X_lo; X_hi; sig_hi; sig_lo]
    lhsT layouts (K x M):
      wma (123, 105): [Wm_hi; Wm_hi; Wm_lo; U_hi; U_hi]   pass-2 main
      wmb (13, 117, 105): rows 9j..9j+8 = U_lo             pass-2 vs state tile
      ws1h/ws1l (126, 126): scan state-propagation lhsT, hi and lo
      wq  (13, 105, 126): per-chunk scan lhsT [wq_hi; wq_hi; wq_lo]
      winit (7, 126) f32: init matmul (y_0 rows 0..2, s_0 rows 117..125)
    """
    import ml_dtypes
    a, b, g = _sigmoid(alpha), _sigmoid(beta), _sigmoid(gamma)
    A, c = _step_mats(a, b, g)
    slots = [(1 + k) % P for k in range(C)]

    Phi = np.zeros((C, 9, 9), np.float64)
    w = np.zeros((C, C, 9), np.float64)
    cur = np.eye(9)
    for k in range(C):
        i = slots[k]
        if k > 0:
            w[k, :k] = w[k - 1, :k] @ A[i].T
        w[k, k] = c[i]
        cur = A[i] @ cur
        Phi[k] = cur
    T = Phi[C - 1]
    V = w[C - 1].T.copy()  # (9, C)

    Wm = np.zeros((C, 105), np.float64)   # X-coefficient block of lhsT
    U = np.zeros((9, 105), np.float64)    # sigma-coefficient block of lhsT
    for k in range(C):
        sel = [0, 1, 2 + slots[k]]
        U[:, 3 * k:3 * k + 3] = Phi[k][sel].T
        for j in range(k + 1):
            Wm[j, 3 * k:3 * k + 3] = w[k, j][sel]

    Tpow = [np.eye(9)]
    for _ in range(G + 1):
        Tpow.append(T @ Tpow[-1])

    ws1 = np.zeros((126, 126), np.float64)
    for j in range(G + 1):
        ws1[117:126, 9 * j:9 * j + 9] = Tpow[j].T
    wqv = np.zeros((G, C, 126), np.float64)
    for i in range(G):
        for j in range(i + 1, G + 1):
            wqv[i, :, 9 * j:9 * j + 9] = (Tpow[j - 1 - i] @ V).T

    winit = np.zeros((7, 126), np.float64)
    winit[0, 0] = 1.0
    winit[0, 1] = -1.0
    winit[1, 1] = 1.0
    winit[0, 117] = 1.0
    winit[0, 118] = -1.0
    winit[1, 118] = 1.0
    for j in range(P):
        winit[j, 119 + j] += 1.0
        winit[0, 119 + j] += -1.0

    Wm_hi, Wm_lo = _hi_lo(Wm)
    U_hi, U_lo = _hi_lo(U)
    ws1_hi, ws1_lo = _hi_lo(ws1)
    wq_hi, wq_lo = _hi_lo(wqv)

    bf = ml_dtypes.bfloat16
    wma = np.zeros((123, 105), bf)
    wma[0:35] = Wm_hi
    wma[35:70] = Wm_hi
    wma[70:105] = Wm_lo
    wma[105:114] = U_hi
    wma[114:123] = U_hi
    wmb = np.zeros((G, 117, 105), bf)
    for j in range(G):
        wmb[j, 9 * j:9 * j + 9] = U_lo
    wq = np.zeros((G, 105, 126), bf)
    wq[:, 0:35] = wq_hi
    wq[:, 35:70] = wq_hi
    wq[:, 70:105] = wq_lo

    return dict(wma=wma, wmb=wmb, ws1h=ws1_hi, ws1l=ws1_lo, wq=wq,
                winit=winit.astype(np.float32))


def build_bass(bl=BL):
    """Build the per-core Bass module (SPMD: same module, sharded inputs)."""
    import concourse.bacc as bacc
    import concourse.mybir as mybir
    from concourse.tile import TileContext

    BF = mybir.dt.bfloat16
    F32 = mybir.dt.float32
    nhalf = min(NHALF, bl)
    nh = (bl + nhalf - 1) // nhalf

    nc = bacc.Bacc(None, target_bir_lowering=False, debug=False)
    xhl = nc.declare_dram_parameter("xhl", [2, L, bl], BF, isOutput=False)
    x0 = nc.declare_dram_parameter("x0", [7, bl], F32, isOutput=False)
    wma_d = nc.declare_dram_parameter("wma", [123, 105], BF, isOutput=False)
    wmb_d = nc.declare_dram_parameter("wmb", [G, 117, 105], BF, isOutput=False)
    ws1h_d = nc.declare_dram_parameter("ws1h", [126, 126], BF, isOutput=False)
    ws1l_d = nc.declare_dram_parameter("ws1l", [126, 126], BF, isOutput=False)
    wq_d = nc.declare_dram_parameter("wq", [G, 105, 126], BF, isOutput=False)
    winit_d = nc.declare_dram_parameter("winit", [7, 126], F32, isOutput=False)
    out_d = nc.declare_dram_parameter("out", [3 * L, bl], F32, isOutput=True)

    with TileContext(nc) as tc:
        with (
            tc.tile_pool(name="consts", bufs=1) as consts,
            tc.tile_pool(name="xpool", bufs=2 * G) as xpool,
            tc.tile_pool(name="spool", bufs=3) as spool,
            tc.tile_pool(name="tpool", bufs=2) as tpool,
            tc.tile_pool(name="ypool", bufs=6) as ypool,
            tc.tile_pool(name="ypsum", bufs=3, space="PSUM") as ypsum,
            tc.tile_pool(name="spsum", bufs=1, space="PSUM") as spsum,
        ):
            wma = consts.tile([123, 105], BF)
            nc.scalar.dma_start(out=wma[:], in_=wma_d[:])
            wmb = consts.tile([117, G * 105], BF)
            ws1h = consts.tile([126, 126], BF)
            nc.scalar.dma_start(out=ws1h[:], in_=ws1h_d[:])
            ws1l = consts.tile([126, 126], BF)
            nc.scalar.dma_start(out=ws1l[:], in_=ws1l_d[:])
            wq = consts.tile([105, G * 126], BF)
            for i in range(G):
                nc.scalar.dma_start(out=wq[:, i * 126:(i + 1) * 126], in_=wq_d[i])
                nc.scalar.dma_start(out=wmb[:, i * 105:(i + 1) * 105], in_=wmb_d[i])
            winit = consts.tile([7, 126], F32)
            nc.scalar.dma_start(out=winit[:], in_=winit_d[:])
            xinit = consts.tile([7, bl], F32)
            nc.scalar.dma_start(out=xinit[:], in_=x0[:])

            # --- init: y_0 rows and s_0 state (zeros elsewhere by construction)
            ip = spsum.tile([126, bl], F32, tag="sp")
            for h in range(nh):
                hs = slice(h * nhalf, (h + 1) * nhalf)
                nc.tensor.matmul(ip[:, hs], lhsT=winit[:], rhs=xinit[:, hs],
                                 start=True, stop=True)
            y0 = ypool.tile([3, bl], F32, tag="y0")
            nc.vector.tensor_copy(out=y0[:], in_=ip[0:3, :])
            nc.scalar.dma_start(out=out_d[0:3, :], in_=y0[:])

            def split_state(psum_tile):
                """psum (126, bl) f32 -> sbuf (126, 2*bl) bf16 [hi | lo]."""
                shl = spool.tile([126, 2 * bl], BF, tag="sprev")
                nc.vector.tensor_copy(out=shl[:, 0:bl], in_=psum_tile[:])
                res = tpool.tile([126, bl], F32, tag="res")
                nc.vector.tensor_sub(out=res[:], in0=psum_tile[:],
                                     in1=shl[:, 0:bl])
                nc.vector.tensor_copy(out=shl[:, bl:2 * bl], in_=res[:])
                return shl

            sprev = split_state(ip)

            for g_ in range(NG):
                xg = []
                for i in range(G):
                    t0 = 1 + C * (G * g_ + i)
                    xt = xpool.tile([123, bl], BF, tag="xg")
                    nc.scalar.dma_start(out=xt[0:C, :], in_=xhl[0, t0:t0 + C, :])
                    nc.scalar.dma_start(out=xt[C:2 * C, :],
                                        in_=xhl[1, t0:t0 + C, :])
                    # duplicate X_hi locally (SBUF->SBUF, off the HBM path)
                    nc.scalar.dma_start(out=xt[2 * C:3 * C, :], in_=xt[0:C, :])
                    xg.append(xt)

                # --- group scan: all 13 chunk-entry states + next group state
                sp = spsum.tile([126, bl], F32, tag="sp")
                for h in range(nh):
                    hs = slice(h * nhalf, (h + 1) * nhalf)
                    nc.tensor.matmul(sp[:, hs], lhsT=ws1h[:],
                                     rhs=sprev[:, h * nhalf:h * nhalf + nhalf],
                                     start=True, stop=False)
                    nc.tensor.matmul(sp[:, hs], lhsT=ws1h[:],
                                     rhs=sprev[:, bl + h * nhalf:bl + h * nhalf + nhalf],
                                     start=False, stop=False)
                    nc.tensor.matmul(sp[:, hs], lhsT=ws1l[:],
                                     rhs=sprev[:, h * nhalf:h * nhalf + nhalf],
                                     start=False, stop=False)
                    for i in range(G):
                        nc.tensor.matmul(sp[:, hs],
                                         lhsT=wq[:, i * 126:(i + 1) * 126],
                                         rhs=xg[i][0:105, hs],
                                         start=False, stop=(i == G - 1))
                sprev_g = split_state(sp)
                # scatter sigma hi/lo into rows 105..113 / 114..122
                for i in range(G):
                    nc.sync.dma_start(out=xg[i][105:114, :],
                                      in_=sprev_g[9 * i:9 * i + 9, 0:bl])
                    nc.sync.dma_start(out=xg[i][114:123, :],
                                      in_=sprev_g[9 * i:9 * i + 9, bl:2 * bl])

                # --- pass 2: per-chunk outputs
                for i in range(G):
                    r0 = 3 * (1 + C * (G * g_ + i))
                    yp = ypsum.tile([105, bl], F32, tag="yp")
                    for h in range(nh):
                        hs = slice(h * nhalf, (h + 1) * nhalf)
                        nc.tensor.matmul(yp[:, hs], lhsT=wma[:],
                                         rhs=xg[i][0:123, hs],
                                         start=True, stop=False)
                    for h in range(nh):
                        hs = slice(h * nhalf, (h + 1) * nhalf)
                        nc.tensor.matmul(yp[:, hs],
                                         lhsT=wmb[:, i * 105:(i + 1) * 105],
                                         rhs=sprev_g[0:117, h * nhalf:h * nhalf + nhalf],
                                         start=False, stop=True)
                    ysb = ypool.tile([105, bl], F32, tag="ysb")
                    nc.vector.tensor_copy(out=ysb[:], in_=yp[:])
                    nc.scalar.dma_start(out=out_d[r0:r0 + 105, :], in_=ysb[:])
                sprev = sprev_g
    nc.compile()
    return nc


def _prep_inputs(x, alpha, beta, gamma):
    import ml_dtypes
    bf = ml_dtypes.bfloat16
    xs = np.asarray(x, dtype=np.float32).reshape(B, L)
    coeffs = _build_coeffs(float(alpha), float(beta), float(gamma))
    in_maps = []
    for m in range(NCORES):
        xT_m = np.ascontiguousarray(xs[m * BL:(m + 1) * BL].T)  # (L, BL) f32
        hi = xT_m.astype(bf)
        lo = (xT_m - hi.astype(np.float32)).astype(bf)
        xhl = np.empty((2, L, BL), bf)
        xhl[0] = hi
        xhl[1] = lo
        x0 = np.ascontiguousarray(xT_m[0:7])                    # (7, BL) f32
        in_maps.append({"xhl": xhl, "x0": x0, **coeffs})
    return in_maps


LAST_RESULT = None  # BassKernelResults of the most recent kernel() call


def kernel(x, alpha, beta, gamma):
    global LAST_RESULT
    from concourse.bass_utils import run_bass_kernel_spmd

    nc = build_bass(BL)
    in_maps = _prep_inputs(x, alpha, beta, gamma)
    res = run_bass_kernel_spmd(nc, in_maps, core_ids=list(range(NCORES)))
    LAST_RESULT = res
    outs = [r["out"] for r in res.results]          # each (3L, BL) float32
    y = np.empty((B, L, 3), np.float32)
    for m in range(NCORES):
        y[m * BL:(m + 1) * BL] = outs[m].T.reshape(BL, L, 3)
    return y



# revision 2
# speedup vs baseline: 1.2847x; 1.2847x over previous
"""Holt-Winters exponential smoothing (level/trend/seasonal, P=7) on 8 Trainium2
NeuronCores — v2: bf16 throughout, C=105 chunks, channel-planar pass-2.

Math: per-row recurrence is linear in a 9-dim state s=[level,trend,buf0..6]:
s_t = A_{t%7} s_{t-1} + c_{t%7} x_t.  Steps t=1..4095 in 39 chunks of C=105
(105%7==0 so all chunks share one weight set), 3 groups of 13 chunks.
Per chunk the outputs are 3 channel-planar matmuls (K=123 = 105 X rows +
9 sigma_hi + 9 sigma_lo, M=105 steps); chunk-entry states come from a
per-group prefix-scan matmul (13 K=105 matmuls + 2 state matmuls per group).
Only sequential dependency: the 3-link group chain.

Precision: x, weights, outputs all single bf16 (fp32 PSUM accumulation);
the chunk-entry states sigma are kept as bf16 hi+lo pairs (column-stacked)
since scan errors compound across groups.  Validated relL2 ~ 4.5e-3 vs
float64 (gate 2e-2).

Sharding: pure data-parallel over batch (1024 rows/core).  Host computes
the trivial t=0 output column and the 9-row init state, and converts the
bf16 device output back to fp32.
"""

import numpy as np

P = 7
C = 105           # steps per chunk (105 % 7 == 0)
G = 13            # chunks per group
NG = 3            # groups; NG*G*C == L-1
NCH = G * NG      # 39 chunks
L = 4096
B = 8192
NCORES = 8
BL = B // NCORES  # 1024 batch rows per core
NH = 512          # matmul moving-dim tile (one fp32 PSUM bank)


def _sigmoid(z):
    return 1.0 / (1.0 + np.exp(-z))


def _step_mats(a, b, g):
    """A_i (9x9), c_i (9,) for seasonal slot i, float64."""
    A, c = [], []
    for i in range(P):
        col = 2 + i
        Ai = np.zeros((9, 9), np.float64)
        ci = np.zeros(9, np.float64)
        Ai[0, 0] = 1 - a
        Ai[0, 1] = 1 - a
        Ai[0, col] += -a
        Ai[1, 0] = -a * b
        Ai[1, 1] = 1 - a * b
        Ai[1, col] += -a * b
        for j in range(P):
            Ai[2 + j, 2 + j] = 1.0
        Ai[col, :] = 0.0
        Ai[col, 0] = -g * (1 - a)
        Ai[col, 1] = -g * (1 - a)
        Ai[col, col] = g * a + 1 - g
        ci[0] = a
        ci[1] = a * b
        ci[col] = g * (1 - a)
        A.append(Ai)
        c.append(ci)
    return A, c


def _build_coeffs(alpha, beta, gamma):
    """Host-precomputed stationary weight matrices (float64 -> bf16).

    wp (3, 123, 105): pass-2 lhsT per channel: rows 0:105 X-coeffs,
        rows 105:114 and 114:123 both the sigma-coeff block U (applied to
        the hi and lo halves of sigma scattered into the rhs tile).
    ws1 (9, 126): scan state lhsT.  Sigma-tile row layout: rows 0:9 =
        group-end state, rows 9+9i:18+9i = entry state of chunk i.
    wq (13, 105, 126): per-chunk scan lhsT.
    """
    import ml_dtypes
    bf = ml_dtypes.bfloat16
    a, b, g = _sigmoid(alpha), _sigmoid(beta), _sigmoid(gamma)
    A, c = _step_mats(a, b, g)
    slots = [(1 + k) % P for k in range(C)]

    Phi = np.zeros((C, 9, 9), np.float64)
    w = np.zeros((C, C, 9), np.float64)
    cur = np.eye(9)
    for k in range(C):
        i = slots[k]
        if k > 0:
            w[k, :k] = w[k - 1, :k] @ A[i].T
        w[k, k] = c[i]
        cur = A[i] @ cur
        Phi[k] = cur
    T = Phi[C - 1]
    V = w[C - 1].T.copy()                 # (9, C)

    wp = np.zeros((3, 123, 105), np.float64)
    for k in range(C):
        sel = [0, 1, 2 + slots[k]]
        for ch in range(3):
            wp[ch, 105:114, k] = Phi[k][sel[ch]]
            wp[ch, 114:123, k] = Phi[k][sel[ch]]
            for j in range(k + 1):
                wp[ch, j, k] = w[k, j][sel[ch]]

    Tpow = [np.eye(9)]
    for _ in range(G + 1):
        Tpow.append(T @ Tpow[-1])

    ws1 = np.zeros((9, 126), np.float64)
    ws1[:, 0:9] = Tpow[G].T
    for j in range(G):
        ws1[:, 9 + 9 * j:18 + 9 * j] = Tpow[j].T
    wq = np.zeros((G, C, 126), np.float64)
    for i in range(G):
        wq[i, :, 0:9] = (Tpow[G - 1 - i] @ V).T
        for j in range(i + 1, G):
            wq[i, :, 9 + 9 * j:18 + 9 * j] = (Tpow[j - 1 - i] @ V).T

    return dict(wp=wp.astype(bf), ws1=ws1.astype(bf), wq=wq.astype(bf))


def build_bass(bl=BL):
    """Build the per-core Bass module (SPMD: same module, sharded inputs)."""
    import concourse.bacc as bacc
    import concourse.mybir as mybir
    from concourse.tile import TileContext

    BF = mybir.dt.bfloat16
    F32 = mybir.dt.float32
    nh = bl // NH                  # moving-dim halves (2)
    GW = G * bl                    # group tile width (13312)

    nc = bacc.Bacc(None, target_bir_lowering=False, debug=False)
    xin = nc.declare_dram_parameter("xin", [C, NCH * bl], BF, isOutput=False)
    s0_d = nc.declare_dram_parameter("s0", [9, 2 * bl], BF, isOutput=False)
    wp_d = nc.declare_dram_parameter("wp", [3, 123, C], BF, isOutput=False)
    ws1_d = nc.declare_dram_parameter("ws1", [9, 126], BF, isOutput=False)
    wq_d = nc.declare_dram_parameter("wq", [G, C, 126], BF, isOutput=False)
    out_d = nc.declare_dram_parameter("out", [C, NCH * 3 * bl], BF,
                                      isOutput=True)

    with TileContext(nc) as tc:
        with (
            tc.tile_pool(name="consts", bufs=1) as consts,
            tc.tile_pool(name="xpool", bufs=NG) as xpool,
            tc.tile_pool(name="spool", bufs=2) as spool,
            tc.tile_pool(name="tpool", bufs=2) as tpool,
            tc.tile_pool(name="ypool", bufs=4) as ypool,
            tc.tile_pool(name="ypsum", bufs=2, space="PSUM") as ypsum,
            tc.tile_pool(name="spsum", bufs=1, space="PSUM") as spsum,
        ):
            wp = consts.tile([123, 3 * C], BF)
            for ch in range(3):
                nc.sync.dma_start(out=wp[:, ch * C:(ch + 1) * C],
                                  in_=wp_d[ch])
            ws1 = consts.tile([9, 126], BF)
            nc.sync.dma_start(out=ws1[:], in_=ws1_d[:])
            wq = consts.tile([C, G * 126], BF)
            for i in range(G):
                nc.sync.dma_start(out=wq[:, i * 126:(i + 1) * 126],
                                  in_=wq_d[i])
            s0 = consts.tile([9, 2 * bl], BF)
            nc.sync.dma_start(out=s0[:], in_=s0_d[:])

            # group input tiles: rows 0:105 X (13 chunks side by side),
            # rows 105:123 sigma hi/lo scatter region
            xg = []
            for g_ in range(NG):
                xt = xpool.tile([123, GW], BF, tag="xg")
                nc.sync.dma_start(out=xt[0:C, :],
                                  in_=xin[:, g_ * GW:(g_ + 1) * GW])
                xg.append(xt)

            st = s0                     # running state tile (9, 2*bl) hi|lo
            for g_ in range(NG):
                # --- group scan: sigma (126, bl) fp32
                sp = spsum.tile([126, bl], F32, tag="sp")
                for h in range(nh):
                    hs = slice(h * NH, (h + 1) * NH)
                    nc.tensor.matmul(sp[:, hs], lhsT=ws1[:],
                                     rhs=st[:, h * NH:h * NH + NH],
                                     start=True, stop=False)
                    nc.tensor.matmul(sp[:, hs], lhsT=ws1[:],
                                     rhs=st[:, bl + h * NH:bl + h * NH + NH],
                                     start=False, stop=False)
                    for i in range(G):
                        nc.tensor.matmul(sp[:, hs],
                                         lhsT=wq[:, i * 126:(i + 1) * 126],
                                         rhs=xg[g_][0:C, i * bl + h * NH:
                                                    i * bl + h * NH + NH],
                                         start=False, stop=(i == G - 1))
                # split sigma -> bf16 hi|lo (126, 2*bl)
                sg = spool.tile([126, 2 * bl], BF, tag="sg")
                nc.vector.tensor_copy(out=sg[:, 0:bl], in_=sp[:])
                res = tpool.tile([126, bl], F32, tag="res")
                nc.vector.tensor_sub(out=res[:], in0=sp[:], in1=sg[:, 0:bl])
                nc.vector.tensor_copy(out=sg[:, bl:2 * bl], in_=res[:])
                # scatter chunk-entry sigma into x tiles rows 105:123
                for i in range(G):
                    r = slice(9 + 9 * i, 18 + 9 * i)
                    nc.sync.dma_start(
                        out=xg[g_][105:114, i * bl:(i + 1) * bl],
                        in_=sg[r, 0:bl])
                    nc.sync.dma_start(
                        out=xg[g_][114:123, i * bl:(i + 1) * bl],
                        in_=sg[r, bl:2 * bl])
                st = sg[0:9, :]

                # --- pass 2: per-chunk outputs, channel-planar
                for i in range(G):
                    ot = ypool.tile([C, 3 * bl], BF, tag="ot")
                    for h in range(nh):
                        yp = ypsum.tile([C, 3 * NH], F32, tag="yp")
                        for ch in range(3):
                            nc.tensor.matmul(
                                yp[:, ch * NH:(ch + 1) * NH],
                                lhsT=wp[:, ch * C:(ch + 1) * C],
                                rhs=xg[g_][0:123, i * bl + h * NH:
                                           i * bl + h * NH + NH],
                                start=True, stop=True)
                        nc.vector.tensor_copy(
                            out=ot[:, h * 3 * NH:(h + 1) * 3 * NH], in_=yp[:])
                    c0 = (g_ * G + i) * 3 * bl
                    nc.scalar.dma_start(out=out_d[:, c0:c0 + 3 * bl],
                                        in_=ot[:])
    nc.compile()
    return nc


def _prep_inputs(x, alpha, beta, gamma):
    import ml_dtypes
    bf = ml_dtypes.bfloat16
    xs = np.asarray(x, dtype=np.float32).reshape(B, L)
    coeffs = _build_coeffs(float(alpha), float(beta), float(gamma))
    in_maps = []
    for m in range(NCORES):
        xm = xs[m * BL:(m + 1) * BL]                    # (BL, L) f32
        xT = np.ascontiguousarray(xm.T)                 # (L, BL)
        xb = xT.astype(bf)                              # (L, BL) bf16
        # xin[k, 13g+i block] = x[1 + 105*(13g+i) + k]
        xin = np.ascontiguousarray(
            xb[1:L].reshape(NCH, C, BL).transpose(1, 0, 2)).reshape(
                C, NCH * BL)
        s0 = np.zeros((9, BL), np.float32)
        s0[0] = xT[0]
        s0[1] = xT[1] - xT[0]
        for j in range(1, P):
            s0[2 + j] = xT[j] - xT[0]
        s0h = s0.astype(bf)
        s0l = (s0 - s0h.astype(np.float32)).astype(bf)
        s0hl = np.concatenate([s0h, s0l], axis=1)       # (9, 2*BL)
        in_maps.append({"xin": xin, "s0": s0hl, **coeffs})
    return in_maps


LAST_RESULT = None  # BassKernelResults of the most recent kernel() call


def kernel(x, alpha, beta, gamma):
    global LAST_RESULT
    from concourse.bass_utils import run_bass_kernel_spmd

    nc = build_bass(BL)
    in_maps = _prep_inputs(x, alpha, beta, gamma)
    res = run_bass_kernel_spmd(nc, in_maps, core_ids=list(range(NCORES)))
    LAST_RESULT = res
    xs = np.asarray(x, dtype=np.float32).reshape(B, L)
    y = np.empty((B, L, 3), np.float32)
    y[:, 0, 0] = xs[:, 0]
    y[:, 0, 1] = xs[:, 1] - xs[:, 0]
    y[:, 0, 2] = 0.0
    for m in range(NCORES):
        o = res.results[m]["out"]                       # (C, NCH*3*BL) bf16
        # cols per chunk: [h0: lvl|tr|se (3*512)] [h1: lvl|tr|se]
        o = o.reshape(C, NCH, 2, 3, NH).astype(np.float32)
        y[m * BL:(m + 1) * BL, 1:, :] = o.transpose(2, 4, 1, 0, 3).reshape(
            BL, L - 1, 3)
    return y


# revision 3
# speedup vs baseline: 1.3409x; 1.0438x over previous
"""Holt-Winters exponential smoothing (level/trend/seasonal, P=7) on 8 Trainium2
NeuronCores — v4: startup + PSUM-pipeline overhaul.

Math identical to v3 (C=105 chunks, 13/group, 3 groups, channel-planar
pass-2, bf16 weights/x/out, fp32 PSUM, sigma_hi-only pass-2, hi/lo state).
Schedule changes vs v3:
  - all weights + s0 coalesced into ONE dram tensor / ONE dma (v3 lost
    35 us to a 13-dma weight-load stall before the first matmul);
  - input group 0 split across the sync and gpsimd queues so the first
    scan starts ~14 us in;
  - PSUM retiled to one-bank tiles: scan 2 x (126,512), pass-2 6 x
    (105,512) -- a 6-deep matmul->cast pipeline that decouples PE pacing
    from cast turnaround (HAM cold-clock trap);
  - output casts alternate DVE / ACT per tile.
"""

import numpy as np

P = 7
C = 105
G = 13
NG = 3
NCH = G * NG
KS = 114          # pass-2 rhs rows: 105 X + 9 sigma_hi
L = 4096
B = 8192
NCORES = 8
BL = B // NCORES
NH = 512

# wall (weights+s0) column offsets
WP0 = 0           # (114, 315)
WQ0 = 315         # (105, 1638)
WS0 = WP0 + 315 + 1638          # ws1 (9, 126)
S00 = WS0 + 126                 # s0 (9, 2048)
WALLW = S00 + 2 * BL            # 4127


def _sigmoid(z):
    return 1.0 / (1.0 + np.exp(-z))


def _step_mats(a, b, g):
    A, c = [], []
    for i in range(P):
        col = 2 + i
        Ai = np.zeros((9, 9), np.float64)
        ci = np.zeros(9, np.float64)
        Ai[0, 0] = 1 - a
        Ai[0, 1] = 1 - a
        Ai[0, col] += -a
        Ai[1, 0] = -a * b
        Ai[1, 1] = 1 - a * b
        Ai[1, col] += -a * b
        for j in range(P):
            Ai[2 + j, 2 + j] = 1.0
        Ai[col, :] = 0.0
        Ai[col, 0] = -g * (1 - a)
        Ai[col, 1] = -g * (1 - a)
        Ai[col, col] = g * a + 1 - g
        ci[0] = a
        ci[1] = a * b
        ci[col] = g * (1 - a)
        A.append(Ai)
        c.append(ci)
    return A, c


def _build_coeffs(alpha, beta, gamma):
    """Weight blocks in float64; packed into the per-core wall later."""
    a, b, g = _sigmoid(alpha), _sigmoid(beta), _sigmoid(gamma)
    A, c = _step_mats(a, b, g)
    slots = [(1 + k) % P for k in range(C)]

    Phi = np.zeros((C, 9, 9), np.float64)
    w = np.zeros((C, C, 9), np.float64)
    cur = np.eye(9)
    for k in range(C):
        i = slots[k]
        if k > 0:
            w[k, :k] = w[k - 1, :k] @ A[i].T
        w[k, k] = c[i]
        cur = A[i] @ cur
        Phi[k] = cur
    T = Phi[C - 1]
    V = w[C - 1].T.copy()

    wp = np.zeros((KS, 3 * C), np.float64)          # [ch0|ch1|ch2]
    for k in range(C):
        sel = [0, 1, 2 + slots[k]]
        for ch in range(3):
            wp[105:114, ch * C + k] = Phi[k][sel[ch]]
            for j in range(k + 1):
                wp[j, ch * C + k] = w[k, j][sel[ch]]

    Tpow = [np.eye(9)]
    for _ in range(G + 1):
        Tpow.append(T @ Tpow[-1])

    ws1 = np.zeros((9, 126), np.float64)
    ws1[:, 0:9] = Tpow[G].T
    for j in range(G):
        ws1[:, 9 + 9 * j:18 + 9 * j] = Tpow[j].T
    wq = np.zeros((C, G * 126), np.float64)         # [i0|i1|...|i12]
    for i in range(G):
        blk = wq[:, i * 126:(i + 1) * 126]
        blk[:, 0:9] = (Tpow[G - 1 - i] @ V).T
        for j in range(i + 1, G):
            blk[:, 9 + 9 * j:18 + 9 * j] = (Tpow[j - 1 - i] @ V).T

    return wp, wq, ws1


def build_bass(bl=BL):
    import concourse.bacc as bacc
    import concourse.mybir as mybir
    from concourse.tile import TileContext

    BF = mybir.dt.bfloat16
    F32 = mybir.dt.float32
    COPY = mybir.ActivationFunctionType.Copy
    nh = bl // NH
    GW = G * bl
    HALFA = 7 * bl                 # input g0 first piece: chunks 0..6

    nc = bacc.Bacc(None, target_bir_lowering=False, debug=False)
    xin = nc.declare_dram_parameter("xin", [C, NCH * bl], BF, isOutput=False)
    wall_d = nc.declare_dram_parameter("wall", [KS, WALLW], BF,
                                       isOutput=False)
    out_d = nc.declare_dram_parameter("out", [C, NCH * 3 * bl], BF,
                                      isOutput=True)

    with TileContext(nc) as tc:
        with (
            tc.tile_pool(name="consts", bufs=1) as consts,
            tc.tile_pool(name="xpool", bufs=NG) as xpool,
            tc.tile_pool(name="spool", bufs=2) as spool,
            tc.tile_pool(name="ypool", bufs=4) as ypool,
            tc.tile_pool(name="ypsum", bufs=6, space="PSUM") as ypsum,
            tc.tile_pool(name="spsum", bufs=2, space="PSUM") as spsum,
        ):
            cw = consts.tile([KS, WALLW], BF)
            nc.sync.dma_start(out=cw[:], in_=wall_d[:])
            wp = cw[:, WP0:WP0 + 3 * C]
            wq = cw[0:C, WQ0:WQ0 + G * 126]
            ws1 = cw[0:9, WS0:WS0 + 126]
            s0 = cw[0:9, S00:S00 + 2 * bl]

            xg = []
            for g_ in range(NG):
                xt = xpool.tile([KS, GW], BF, tag="xg", name=f"xg{g_}")
                if g_ == 0:
                    nc.sync.dma_start(out=xt[0:C, 0:HALFA],
                                      in_=xin[:, 0:HALFA])
                    nc.gpsimd.dma_start(out=xt[0:C, HALFA:GW],
                                        in_=xin[:, HALFA:GW])
                else:
                    nc.gpsimd.dma_start(out=xt[0:C, :],
                                        in_=xin[:, g_ * GW:(g_ + 1) * GW])
                xg.append(xt)

            state = [(s0[:, 0:bl], s0[:, bl:2 * bl])]

            def scan_split_scatter(g_):
                st_h, st_l = state[g_]
                sps = []
                for h in range(nh):
                    hs = slice(h * NH, (h + 1) * NH)
                    sp = spsum.tile([126, NH], F32, tag="sp",
                                    name=f"sp{g_}_{h}")
                    nc.tensor.matmul(sp[:], lhsT=ws1[:], rhs=st_h[:, hs],
                                     start=True, stop=False)
                    nc.tensor.matmul(sp[:], lhsT=ws1[:], rhs=st_l[:, hs],
                                     start=False, stop=False)
                    for i in range(G):
                        nc.tensor.matmul(sp[:],
                                         lhsT=wq[:, i * 126:(i + 1) * 126],
                                         rhs=xg[g_][0:C, i * bl + h * NH:
                                                    i * bl + h * NH + NH],
                                         start=False, stop=(i == G - 1))
                    sps.append(sp)
                sg = spool.tile([126, bl], BF, tag="sg", name=f"sg{g_}")
                res = spool.tile([9, bl], F32, tag="res", name=f"res{g_}")
                stl = spool.tile([9, bl], BF, tag="stl", name=f"stl{g_}")
                for h in range(nh):
                    hs = slice(h * NH, (h + 1) * NH)
                    nc.scalar.activation(out=sg[:, hs], in_=sps[h][:],
                                         func=COPY)
                    nc.vector.tensor_sub(out=res[:, hs], in0=sps[h][0:9, :],
                                         in1=sg[0:9, hs])
                nc.vector.tensor_copy(out=stl[:], in_=res[:])
                for i in range(G):
                    nc.sync.dma_start(
                        out=xg[g_][105:114, i * bl:(i + 1) * bl],
                        in_=sg[9 + 9 * i:18 + 9 * i, :])
                state.append((sg[0:9, :], stl[:]))

            def pass2_chunk(g_, i):
                ot = ypool.tile([C, 3 * bl], BF, tag="ot", name=f"ot{g_}_{i}")
                k = 0
                for h in range(nh):
                    chs = (0, 1, 2) if h == 0 else (2, 1, 0)
                    for ch in chs:
                        yp = ypsum.tile([C, NH], F32, tag="yp",
                                        name=f"yp{g_}_{i}_{h}_{ch}")
                        nc.tensor.matmul(
                            yp[:],
                            lhsT=wp[:, ch * C:(ch + 1) * C],
                            rhs=xg[g_][0:KS, i * bl + h * NH:
                                       i * bl + h * NH + NH],
                            start=True, stop=True)
                        oc = slice((h * 3 + ch) * NH, (h * 3 + ch + 1) * NH)
                        if k % 2 == 0:
                            nc.vector.tensor_copy(out=ot[:, oc], in_=yp[:])
                        else:
                            nc.scalar.activation(out=ot[:, oc], in_=yp[:],
                                                 func=COPY)
                        k += 1
                c0 = (g_ * G + i) * 3 * bl
                nc.gpsimd.dma_start(out=out_d[:, c0:c0 + 3 * bl], in_=ot[:])

            scan_split_scatter(0)
            for g_ in range(NG):
                for i in range(G):
                    if i == 3 and g_ + 1 < NG:
                        scan_split_scatter(g_ + 1)
                    pass2_chunk(g_, i)
    nc.compile()
    return nc


def _prep_inputs(x, alpha, beta, gamma):
    import ml_dtypes
    bf = ml_dtypes.bfloat16
    xs = np.asarray(x, dtype=np.float32).reshape(B, L)
    wp, wq, ws1 = _build_coeffs(float(alpha), float(beta), float(gamma))
    wall0 = np.zeros((KS, WALLW), np.float32)
    wall0[:, WP0:WP0 + 3 * C] = wp
    wall0[0:C, WQ0:WQ0 + G * 126] = wq
    wall0[0:9, WS0:WS0 + 126] = ws1
    in_maps = []
    for m in range(NCORES):
        xm = xs[m * BL:(m + 1) * BL]
        xT = np.ascontiguousarray(xm.T)
        xb = xT.astype(bf)
        xin = np.ascontiguousarray(
            xb[1:L].reshape(NCH, C, BL).transpose(1, 0, 2)).reshape(
                C, NCH * BL)
        s0 = np.zeros((9, BL), np.float32)
        s0[0] = xT[0]
        s0[1] = xT[1] - xT[0]
        for j in range(1, P):
            s0[2 + j] = xT[j] - xT[0]
        s0h = s0.astype(bf)
        s0l = (s0 - s0h.astype(np.float32)).astype(bf)
        wall = wall0.copy()
        wall[0:9, S00:S00 + BL] = s0h
        wall[0:9, S00 + BL:S00 + 2 * BL] = s0l
        in_maps.append({"xin": xin, "wall": wall.astype(bf)})
    return in_maps


LAST_RESULT = None


def kernel(x, alpha, beta, gamma):
    global LAST_RESULT
    from concourse.bass_utils import run_bass_kernel_spmd

    nc = build_bass(BL)
    in_maps = _prep_inputs(x, alpha, beta, gamma)
    res = run_bass_kernel_spmd(nc, in_maps, core_ids=list(range(NCORES)))
    LAST_RESULT = res
    xs = np.asarray(x, dtype=np.float32).reshape(B, L)
    y = np.empty((B, L, 3), np.float32)
    y[:, 0, 0] = xs[:, 0]
    y[:, 0, 1] = xs[:, 1] - xs[:, 0]
    y[:, 0, 2] = 0.0
    for m in range(NCORES):
        o = res.results[m]["out"]
        o = o.reshape(C, NCH, 2, 3, NH).astype(np.float32)
        y[m * BL:(m + 1) * BL, 1:, :] = o.transpose(2, 4, 1, 0, 3).reshape(
            BL, L - 1, 3)
    return y


# revision 4
# speedup vs baseline: 1.3855x; 1.0333x over previous
"""Holt-Winters exponential smoothing (level/trend/seasonal, P=7) on 8 Trainium2
NeuronCores — v5: fine-grained input loads + tail drain.

Math identical to v3 (C=105 chunks, 13/group, 3 groups, channel-planar
pass-2, bf16 weights/x/out, fp32 PSUM, sigma_hi-only pass-2, hi/lo state).
Schedule changes vs v3:
  - all weights + s0 coalesced into ONE dram tensor / ONE dma (v3 lost
    35 us to a 13-dma weight-load stall before the first matmul);
  - input group 0 split across the sync and gpsimd queues so the first
    scan starts ~14 us in;
  - PSUM retiled to one-bank tiles: scan 2 x (126,512), pass-2 6 x
    (105,512) -- a 6-deep matmul->cast pipeline that decouples PE pacing
    from cast turnaround (HAM cold-clock trap);
  - output casts alternate DVE / ACT per tile.
"""

import numpy as np

P = 7
C = 105
G = 13
NG = 3
NCH = G * NG
KS = 114          # pass-2 rhs rows: 105 X + 9 sigma_hi
L = 4096
B = 8192
NCORES = 8
BL = B // NCORES
NH = 512

# wall (weights+s0) column offsets
WP0 = 0           # (114, 315)
WQ0 = 315         # (105, 1638)
WS0 = WP0 + 315 + 1638          # ws1 (9, 126)
S00 = WS0 + 126                 # s0 (9, 2048)
WALLW = S00 + 2 * BL            # 4127


def _sigmoid(z):
    return 1.0 / (1.0 + np.exp(-z))


def _step_mats(a, b, g):
    A, c = [], []
    for i in range(P):
        col = 2 + i
        Ai = np.zeros((9, 9), np.float64)
        ci = np.zeros(9, np.float64)
        Ai[0, 0] = 1 - a
        Ai[0, 1] = 1 - a
        Ai[0, col] += -a
        Ai[1, 0] = -a * b
        Ai[1, 1] = 1 - a * b
        Ai[1, col] += -a * b
        for j in range(P):
            Ai[2 + j, 2 + j] = 1.0
        Ai[col, :] = 0.0
        Ai[col, 0] = -g * (1 - a)
        Ai[col, 1] = -g * (1 - a)
        Ai[col, col] = g * a + 1 - g
        ci[0] = a
        ci[1] = a * b
        ci[col] = g * (1 - a)
        A.append(Ai)
        c.append(ci)
    return A, c


def _build_coeffs(alpha, beta, gamma):
    """Weight blocks in float64; packed into the per-core wall later."""
    a, b, g = _sigmoid(alpha), _sigmoid(beta), _sigmoid(gamma)
    A, c = _step_mats(a, b, g)
    slots = [(1 + k) % P for k in range(C)]

    Phi = np.zeros((C, 9, 9), np.float64)
    w = np.zeros((C, C, 9), np.float64)
    cur = np.eye(9)
    for k in range(C):
        i = slots[k]
        if k > 0:
            w[k, :k] = w[k - 1, :k] @ A[i].T
        w[k, k] = c[i]
        cur = A[i] @ cur
        Phi[k] = cur
    T = Phi[C - 1]
    V = w[C - 1].T.copy()

    wp = np.zeros((KS, 3 * C), np.float64)          # [ch0|ch1|ch2]
    for k in range(C):
        sel = [0, 1, 2 + slots[k]]
        for ch in range(3):
            wp[105:114, ch * C + k] = Phi[k][sel[ch]]
            for j in range(k + 1):
                wp[j, ch * C + k] = w[k, j][sel[ch]]

    Tpow = [np.eye(9)]
    for _ in range(G + 1):
        Tpow.append(T @ Tpow[-1])

    ws1 = np.zeros((9, 126), np.float64)
    ws1[:, 0:9] = Tpow[G].T
    for j in range(G):
        ws1[:, 9 + 9 * j:18 + 9 * j] = Tpow[j].T
    wq = np.zeros((C, G * 126), np.float64)         # [i0|i1|...|i12]
    for i in range(G):
        blk = wq[:, i * 126:(i + 1) * 126]
        blk[:, 0:9] = (Tpow[G - 1 - i] @ V).T
        for j in range(i + 1, G):
            blk[:, 9 + 9 * j:18 + 9 * j] = (Tpow[j - 1 - i] @ V).T

    return wp, wq, ws1


def build_bass(bl=BL):
    import concourse.bacc as bacc
    import concourse.mybir as mybir
    from concourse.tile import TileContext

    BF = mybir.dt.bfloat16
    F32 = mybir.dt.float32
    COPY = mybir.ActivationFunctionType.Copy
    nh = bl // NH
    GW = G * bl

    nc = bacc.Bacc(None, target_bir_lowering=False, debug=False)
    xin = nc.declare_dram_parameter("xin", [C, NCH * bl], BF, isOutput=False)
    wall_d = nc.declare_dram_parameter("wall", [KS, WALLW], BF,
                                       isOutput=False)
    out_d = nc.declare_dram_parameter("out", [C, NCH * 3 * bl], BF,
                                      isOutput=True)

    with TileContext(nc) as tc:
        with (
            tc.tile_pool(name="consts", bufs=1) as consts,
            tc.tile_pool(name="xpool", bufs=NG) as xpool,
            tc.tile_pool(name="spool", bufs=2) as spool,
            tc.tile_pool(name="ypool", bufs=4) as ypool,
            tc.tile_pool(name="ypsum", bufs=6, space="PSUM") as ypsum,
            tc.tile_pool(name="spsum", bufs=2, space="PSUM") as spsum,
        ):
            cw = consts.tile([KS, WALLW], BF)
            nc.sync.dma_start(out=cw[:], in_=wall_d[:])
            wp = cw[:, WP0:WP0 + 3 * C]
            wq = cw[0:C, WQ0:WQ0 + G * 126]
            ws1 = cw[0:9, WS0:WS0 + 126]
            s0 = cw[0:9, S00:S00 + 2 * bl]

            # fine-grained input loads: 2-chunk pieces for g0 (fast start,
            # 4KB partition lines), 3-4-chunk pieces for g1/g2; spread over
            # the sync and gpsimd queues to engage all engines early.
            xg = []
            pieces = {0: ([(0, 2, 0), (2, 4, 1), (4, 6, 0), (6, 8, 1),
                           (8, 10, 0), (10, 12, 1), (12, 13, 0)]),
                      1: ([(0, 4, 0), (4, 8, 0), (8, 11, 0), (11, 13, 0)]),
                      2: ([(0, 4, 1), (4, 8, 1), (8, 11, 1), (11, 13, 1)])}
            for g_ in range(NG):
                xt = xpool.tile([KS, GW], BF, tag="xg", name=f"xg{g_}")
                for (a, b, q) in pieces[g_]:
                    src = xin[:, g_ * GW + a * bl:g_ * GW + b * bl]
                    dst = xt[0:C, a * bl:b * bl]
                    if q == 0:
                        nc.sync.dma_start(out=dst, in_=src)
                    else:
                        nc.gpsimd.dma_start(out=dst, in_=src)
                xg.append(xt)

            state = [(s0[:, 0:bl], s0[:, bl:2 * bl])]

            def scan_split_scatter(g_):
                st_h, st_l = state[g_]
                sps = []
                for h in range(nh):
                    hs = slice(h * NH, (h + 1) * NH)
                    sp = spsum.tile([126, NH], F32, tag="sp",
                                    name=f"sp{g_}_{h}")
                    nc.tensor.matmul(sp[:], lhsT=ws1[:], rhs=st_h[:, hs],
                                     start=True, stop=False)
                    nc.tensor.matmul(sp[:], lhsT=ws1[:], rhs=st_l[:, hs],
                                     start=False, stop=False)
                    for i in range(G):
                        nc.tensor.matmul(sp[:],
                                         lhsT=wq[:, i * 126:(i + 1) * 126],
                                         rhs=xg[g_][0:C, i * bl + h * NH:
                                                    i * bl + h * NH + NH],
                                         start=False, stop=(i == G - 1))
                    sps.append(sp)
                sg = spool.tile([126, bl], BF, tag="sg", name=f"sg{g_}")
                res = spool.tile([9, bl], F32, tag="res", name=f"res{g_}")
                stl = spool.tile([9, bl], BF, tag="stl", name=f"stl{g_}")
                for h in range(nh):
                    hs = slice(h * NH, (h + 1) * NH)
                    nc.scalar.activation(out=sg[:, hs], in_=sps[h][:],
                                         func=COPY)
                    nc.vector.tensor_sub(out=res[:, hs], in0=sps[h][0:9, :],
                                         in1=sg[0:9, hs])
                nc.vector.tensor_copy(out=stl[:], in_=res[:])
                for i in range(G):
                    nc.sync.dma_start(
                        out=xg[g_][105:114, i * bl:(i + 1) * bl],
                        in_=sg[9 + 9 * i:18 + 9 * i, :])
                state.append((sg[0:9, :], stl[:]))

            def pass2_chunk(g_, i):
                ot = ypool.tile([C, 3 * bl], BF, tag="ot", name=f"ot{g_}_{i}")
                k = 0
                for h in range(nh):
                    chs = (0, 1, 2) if h == 0 else (2, 1, 0)
                    for ch in chs:
                        yp = ypsum.tile([C, NH], F32, tag="yp",
                                        name=f"yp{g_}_{i}_{h}_{ch}")
                        nc.tensor.matmul(
                            yp[:],
                            lhsT=wp[:, ch * C:(ch + 1) * C],
                            rhs=xg[g_][0:KS, i * bl + h * NH:
                                       i * bl + h * NH + NH],
                            start=True, stop=True)
                        oc = slice((h * 3 + ch) * NH, (h * 3 + ch + 1) * NH)
                        if k % 2 == 0:
                            nc.vector.tensor_copy(out=ot[:, oc], in_=yp[:])
                        else:
                            nc.scalar.activation(out=ot[:, oc], in_=yp[:],
                                                 func=COPY)
                        k += 1
                c0 = (g_ * G + i) * 3 * bl
                if g_ == NG - 1 and i % 2 == 1:
                    nc.sync.dma_start(out=out_d[:, c0:c0 + 3 * bl], in_=ot[:])
                else:
                    nc.gpsimd.dma_start(out=out_d[:, c0:c0 + 3 * bl],
                                        in_=ot[:])

            scan_split_scatter(0)
            for g_ in range(NG):
                for i in range(G):
                    if i == 3 and g_ + 1 < NG:
                        scan_split_scatter(g_ + 1)
                    pass2_chunk(g_, i)
    nc.compile()
    return nc


def _prep_inputs(x, alpha, beta, gamma):
    import ml_dtypes
    bf = ml_dtypes.bfloat16
    xs = np.asarray(x, dtype=np.float32).reshape(B, L)
    wp, wq, ws1 = _build_coeffs(float(alpha), float(beta), float(gamma))
    wall0 = np.zeros((KS, WALLW), np.float32)
    wall0[:, WP0:WP0 + 3 * C] = wp
    wall0[0:C, WQ0:WQ0 + G * 126] = wq
    wall0[0:9, WS0:WS0 + 126] = ws1
    in_maps = []
    for m in range(NCORES):
        xm = xs[m * BL:(m + 1) * BL]
        xT = np.ascontiguousarray(xm.T)
        xb = xT.astype(bf)
        xin = np.ascontiguousarray(
            xb[1:L].reshape(NCH, C, BL).transpose(1, 0, 2)).reshape(
                C, NCH * BL)
        s0 = np.zeros((9, BL), np.float32)
        s0[0] = xT[0]
        s0[1] = xT[1] - xT[0]
        for j in range(1, P):
            s0[2 + j] = xT[j] - xT[0]
        s0h = s0.astype(bf)
        s0l = (s0 - s0h.astype(np.float32)).astype(bf)
        wall = wall0.copy()
        wall[0:9, S00:S00 + BL] = s0h
        wall[0:9, S00 + BL:S00 + 2 * BL] = s0l
        in_maps.append({"xin": xin, "wall": wall.astype(bf)})
    return in_maps


LAST_RESULT = None


def kernel(x, alpha, beta, gamma):
    global LAST_RESULT
    from concourse.bass_utils import run_bass_kernel_spmd

    nc = build_bass(BL)
    in_maps = _prep_inputs(x, alpha, beta, gamma)
    res = run_bass_kernel_spmd(nc, in_maps, core_ids=list(range(NCORES)))
    LAST_RESULT = res
    xs = np.asarray(x, dtype=np.float32).reshape(B, L)
    y = np.empty((B, L, 3), np.float32)
    y[:, 0, 0] = xs[:, 0]
    y[:, 0, 1] = xs[:, 1] - xs[:, 0]
    y[:, 0, 2] = 0.0
    for m in range(NCORES):
        o = res.results[m]["out"]
        o = o.reshape(C, NCH, 2, 3, NH).astype(np.float32)
        y[m * BL:(m + 1) * BL, 1:, :] = o.transpose(2, 4, 1, 0, 3).reshape(
            BL, L - 1, 3)
    return y


# revision 6
# speedup vs baseline: 1.4080x; 1.0162x over previous
"""Holt-Winters exponential smoothing (level/trend/seasonal, P=7) on 8 Trainium2
NeuronCores — v6: startup trim (split wall dma, queue order).

Math identical to v3 (C=105 chunks, 13/group, 3 groups, channel-planar
pass-2, bf16 weights/x/out, fp32 PSUM, sigma_hi-only pass-2, hi/lo state).
Schedule changes vs v3:
  - all weights + s0 coalesced into ONE dram tensor / ONE dma (v3 lost
    35 us to a 13-dma weight-load stall before the first matmul);
  - input group 0 split across the sync and gpsimd queues so the first
    scan starts ~14 us in;
  - PSUM retiled to one-bank tiles: scan 2 x (126,512), pass-2 6 x
    (105,512) -- a 6-deep matmul->cast pipeline that decouples PE pacing
    from cast turnaround (HAM cold-clock trap);
  - output casts alternate DVE / ACT per tile.
"""

import numpy as np

P = 7
C = 105
G = 13
NG = 3
NCH = G * NG
KS = 114          # pass-2 rhs rows: 105 X + 9 sigma_hi
L = 4096
B = 8192
NCORES = 8
BL = B // NCORES
NH = 512

# wall (weights+s0) column offsets
WP0 = 0           # (114, 315)
WQ0 = 315         # (105, 1638)
WS0 = WP0 + 315 + 1638          # ws1 (9, 126)
S00 = WS0 + 126                 # s0 (9, 2048)
WALLW = S00 + 2 * BL            # 4127


def _sigmoid(z):
    return 1.0 / (1.0 + np.exp(-z))


def _step_mats(a, b, g):
    A, c = [], []
    for i in range(P):
        col = 2 + i
        Ai = np.zeros((9, 9), np.float64)
        ci = np.zeros(9, np.float64)
        Ai[0, 0] = 1 - a
        Ai[0, 1] = 1 - a
        Ai[0, col] += -a
        Ai[1, 0] = -a * b
        Ai[1, 1] = 1 - a * b
        Ai[1, col] += -a * b
        for j in range(P):
            Ai[2 + j, 2 + j] = 1.0
        Ai[col, :] = 0.0
        Ai[col, 0] = -g * (1 - a)
        Ai[col, 1] = -g * (1 - a)
        Ai[col, col] = g * a + 1 - g
        ci[0] = a
        ci[1] = a * b
        ci[col] = g * (1 - a)
        A.append(Ai)
        c.append(ci)
    return A, c


def _build_coeffs(alpha, beta, gamma):
    """Weight blocks in float64; packed into the per-core wall later."""
    a, b, g = _sigmoid(alpha), _sigmoid(beta), _sigmoid(gamma)
    A, c = _step_mats(a, b, g)
    slots = [(1 + k) % P for k in range(C)]

    Phi = np.zeros((C, 9, 9), np.float64)
    w = np.zeros((C, C, 9), np.float64)
    cur = np.eye(9)
    for k in range(C):
        i = slots[k]
        if k > 0:
            w[k, :k] = w[k - 1, :k] @ A[i].T
        w[k, k] = c[i]
        cur = A[i] @ cur
        Phi[k] = cur
    T = Phi[C - 1]
    V = w[C - 1].T.copy()

    wp = np.zeros((KS, 3 * C), np.float64)          # [ch0|ch1|ch2]
    for k in range(C):
        sel = [0, 1, 2 + slots[k]]
        for ch in range(3):
            wp[105:114, ch * C + k] = Phi[k][sel[ch]]
            for j in range(k + 1):
                wp[j, ch * C + k] = w[k, j][sel[ch]]

    Tpow = [np.eye(9)]
    for _ in range(G + 1):
        Tpow.append(T @ Tpow[-1])

    ws1 = np.zeros((9, 126), np.float64)
    ws1[:, 0:9] = Tpow[G].T
    for j in range(G):
        ws1[:, 9 + 9 * j:18 + 9 * j] = Tpow[j].T
    wq = np.zeros((C, G * 126), np.float64)         # [i0|i1|...|i12]
    for i in range(G):
        blk = wq[:, i * 126:(i + 1) * 126]
        blk[:, 0:9] = (Tpow[G - 1 - i] @ V).T
        for j in range(i + 1, G):
            blk[:, 9 + 9 * j:18 + 9 * j] = (Tpow[j - 1 - i] @ V).T

    return wp, wq, ws1


def build_bass(bl=BL):
    import concourse.bacc as bacc
    import concourse.mybir as mybir
    from concourse.tile import TileContext

    BF = mybir.dt.bfloat16
    F32 = mybir.dt.float32
    COPY = mybir.ActivationFunctionType.Copy
    nh = bl // NH
    GW = G * bl

    nc = bacc.Bacc(None, target_bir_lowering=False, debug=False)
    xin = nc.declare_dram_parameter("xin", [C, NCH * bl], BF, isOutput=False)
    wall_d = nc.declare_dram_parameter("wall", [KS, WALLW], BF,
                                       isOutput=False)
    out_d = nc.declare_dram_parameter("out", [C, NCH * 3 * bl], BF,
                                      isOutput=True)

    with TileContext(nc) as tc:
        with (
            tc.tile_pool(name="consts", bufs=1) as consts,
            tc.tile_pool(name="xpool", bufs=NG) as xpool,
            tc.tile_pool(name="spool", bufs=2) as spool,
            tc.tile_pool(name="ypool", bufs=4) as ypool,
            tc.tile_pool(name="ypsum", bufs=6, space="PSUM") as ypsum,
            tc.tile_pool(name="spsum", bufs=2, space="PSUM") as spsum,
        ):
            cw = consts.tile([KS, WALLW], BF)
            # s0 block first (tiny, gates the first scan matmuls), then the
            # weight block; rows 9:114 of the s0 region are never read, so
            # skip them instead of DMAing 430KB of zero padding.
            nc.sync.dma_start(out=cw[0:9, S00:WALLW], in_=wall_d[0:9, S00:WALLW])
            nc.sync.dma_start(out=cw[:, 0:S00], in_=wall_d[:, 0:S00])
            wp = cw[:, WP0:WP0 + 3 * C]
            wq = cw[0:C, WQ0:WQ0 + G * 126]
            ws1 = cw[0:9, WS0:WS0 + 126]
            s0 = cw[0:9, S00:S00 + 2 * bl]

            # fine-grained input loads: 2-chunk pieces for g0 (fast start,
            # 4KB partition lines), 3-4-chunk pieces for g1/g2; spread over
            # the sync and gpsimd queues to engage all engines early.
            xg = []
            pieces = {0: ([(0, 2, 1), (2, 4, 1), (4, 6, 1), (6, 8, 1),
                           (8, 10, 1), (10, 12, 1), (12, 13, 1)]),
                      1: ([(0, 4, 0), (4, 8, 0), (8, 11, 0), (11, 13, 0)]),
                      2: ([(0, 4, 1), (4, 8, 1), (8, 11, 1), (11, 13, 1)])}
            for g_ in range(NG):
                xt = xpool.tile([KS, GW], BF, tag="xg", name=f"xg{g_}")
                for (a, b, q) in pieces[g_]:
                    src = xin[:, g_ * GW + a * bl:g_ * GW + b * bl]
                    dst = xt[0:C, a * bl:b * bl]
                    if q == 0:
                        nc.sync.dma_start(out=dst, in_=src)
                    else:
                        nc.gpsimd.dma_start(out=dst, in_=src)
                xg.append(xt)

            state = [(s0[:, 0:bl], s0[:, bl:2 * bl])]

            def scan_split_scatter(g_):
                st_h, st_l = state[g_]
                sps = []
                for h in range(nh):
                    hs = slice(h * NH, (h + 1) * NH)
                    sp = spsum.tile([126, NH], F32, tag="sp",
                                    name=f"sp{g_}_{h}")
                    nc.tensor.matmul(sp[:], lhsT=ws1[:], rhs=st_h[:, hs],
                                     start=True, stop=False)
                    nc.tensor.matmul(sp[:], lhsT=ws1[:], rhs=st_l[:, hs],
                                     start=False, stop=False)
                    for i in range(G):
                        nc.tensor.matmul(sp[:],
                                         lhsT=wq[:, i * 126:(i + 1) * 126],
                                         rhs=xg[g_][0:C, i * bl + h * NH:
                                                    i * bl + h * NH + NH],
                                         start=False, stop=(i == G - 1))
                    sps.append(sp)
                sg = spool.tile([126, bl], BF, tag="sg", name=f"sg{g_}")
                res = spool.tile([9, bl], F32, tag="res", name=f"res{g_}")
                stl = spool.tile([9, bl], BF, tag="stl", name=f"stl{g_}")
                for h in range(nh):
                    hs = slice(h * NH, (h + 1) * NH)
                    nc.scalar.activation(out=sg[:, hs], in_=sps[h][:],
                                         func=COPY)
                    nc.vector.tensor_sub(out=res[:, hs], in0=sps[h][0:9, :],
                                         in1=sg[0:9, hs])
                nc.vector.tensor_copy(out=stl[:], in_=res[:])
                for i in range(G):
                    nc.sync.dma_start(
                        out=xg[g_][105:114, i * bl:(i + 1) * bl],
                        in_=sg[9 + 9 * i:18 + 9 * i, :])
                state.append((sg[0:9, :], stl[:]))

            def pass2_chunk(g_, i):
                ot = ypool.tile([C, 3 * bl], BF, tag="ot", name=f"ot{g_}_{i}")
                k = 0
                for h in range(nh):
                    chs = (0, 1, 2) if h == 0 else (2, 1, 0)
                    for ch in chs:
                        yp = ypsum.tile([C, NH], F32, tag="yp",
                                        name=f"yp{g_}_{i}_{h}_{ch}")
                        nc.tensor.matmul(
                            yp[:],
                            lhsT=wp[:, ch * C:(ch + 1) * C],
                            rhs=xg[g_][0:KS, i * bl + h * NH:
                                       i * bl + h * NH + NH],
                            start=True, stop=True)
                        oc = slice((h * 3 + ch) * NH, (h * 3 + ch + 1) * NH)
                        if k % 2 == 0:
                            nc.vector.tensor_copy(out=ot[:, oc], in_=yp[:])
                        else:
                            nc.scalar.activation(out=ot[:, oc], in_=yp[:],
                                                 func=COPY)
                        k += 1
                c0 = (g_ * G + i) * 3 * bl
                if g_ == NG - 1 and i % 2 == 1:
                    nc.sync.dma_start(out=out_d[:, c0:c0 + 3 * bl], in_=ot[:])
                else:
                    nc.gpsimd.dma_start(out=out_d[:, c0:c0 + 3 * bl],
                                        in_=ot[:])

            scan_split_scatter(0)
            for g_ in range(NG):
                for i in range(G):
                    if i == 3 and g_ + 1 < NG:
                        scan_split_scatter(g_ + 1)
                    pass2_chunk(g_, i)
    nc.compile()
    return nc


def _prep_inputs(x, alpha, beta, gamma):
    import ml_dtypes
    bf = ml_dtypes.bfloat16
    xs = np.asarray(x, dtype=np.float32).reshape(B, L)
    wp, wq, ws1 = _build_coeffs(float(alpha), float(beta), float(gamma))
    wall0 = np.zeros((KS, WALLW), np.float32)
    wall0[:, WP0:WP0 + 3 * C] = wp
    wall0[0:C, WQ0:WQ0 + G * 126] = wq
    wall0[0:9, WS0:WS0 + 126] = ws1
    in_maps = []
    for m in range(NCORES):
        xm = xs[m * BL:(m + 1) * BL]
        xT = np.ascontiguousarray(xm.T)
        xb = xT.astype(bf)
        xin = np.ascontiguousarray(
            xb[1:L].reshape(NCH, C, BL).transpose(1, 0, 2)).reshape(
                C, NCH * BL)
        s0 = np.zeros((9, BL), np.float32)
        s0[0] = xT[0]
        s0[1] = xT[1] - xT[0]
        for j in range(1, P):
            s0[2 + j] = xT[j] - xT[0]
        s0h = s0.astype(bf)
        s0l = (s0 - s0h.astype(np.float32)).astype(bf)
        wall = wall0.copy()
        wall[0:9, S00:S00 + BL] = s0h
        wall[0:9, S00 + BL:S00 + 2 * BL] = s0l
        in_maps.append({"xin": xin, "wall": wall.astype(bf)})
    return in_maps


LAST_RESULT = None

def _ensure_ntff_hook():
    """If BASS_TRACE is set but this environment lacks antenv.axon_hooks
    (concourse imports it under axon when tracing), provide it -- registered
    from the injected libaxon_pjrt.so when available, else a no-op so
    run_bass_kernel_spmd degrades to an untraced run instead of crashing."""
    import importlib.util
    try:
        if importlib.util.find_spec("antenv.axon_hooks") is not None:
            return
    except (ImportError, ModuleNotFoundError, ValueError):
        pass
    import contextlib
    import ctypes
    import sys
    import types

    mod = types.ModuleType("antenv.axon_hooks")
    mod._hook = None
    mod.set_axon_ntff_profile_hook = lambda h: setattr(mod, "_hook", h)
    mod.get_axon_ntff_profile_hook = lambda: mod._hook
    sys.modules["antenv.axon_hooks"] = mod
    try:
        import antenv
        antenv.axon_hooks = mod
    except ImportError:
        pass
    try:
        lib = ctypes.CDLL("/opt/axon/libaxon_pjrt.so")
        if not hasattr(lib, "axon_start_nrt_profile"):
            return
        lib.axon_start_nrt_profile.argtypes = [
            ctypes.POINTER(ctypes.c_int64), ctypes.c_size_t]
        lib.axon_start_nrt_profile.restype = ctypes.c_int64
        lib.axon_stop_nrt_profile.argtypes = [ctypes.c_char_p]
        lib.axon_stop_nrt_profile.restype = ctypes.c_int64

        @contextlib.contextmanager
        def _hook(output_dir, device_ids):
            import jax
            jax.devices()
            if device_ids:
                ids = (ctypes.c_int64 * len(device_ids))(*device_ids)
                rc = lib.axon_start_nrt_profile(ids, len(device_ids))
            else:
                rc = lib.axon_start_nrt_profile(None, 0)
            if rc != 0:
                raise RuntimeError(f"axon_start_nrt_profile rc={rc}")
            try:
                yield
            finally:
                lib.axon_stop_nrt_profile(str(output_dir).encode())

        mod.set_axon_ntff_profile_hook(_hook)
    except OSError:
        pass



def kernel(x, alpha, beta, gamma):
    global LAST_RESULT
    _ensure_ntff_hook()
    from concourse.bass_utils import run_bass_kernel_spmd

    nc = build_bass(BL)
    in_maps = _prep_inputs(x, alpha, beta, gamma)
    res = run_bass_kernel_spmd(nc, in_maps, core_ids=list(range(NCORES)))
    LAST_RESULT = res
    xs = np.asarray(x, dtype=np.float32).reshape(B, L)
    y = np.empty((B, L, 3), np.float32)
    y[:, 0, 0] = xs[:, 0]
    y[:, 0, 1] = xs[:, 1] - xs[:, 0]
    y[:, 0, 2] = 0.0
    for m in range(NCORES):
        o = res.results[m]["out"]
        o = o.reshape(C, NCH, 2, 3, NH).astype(np.float32)
        y[m * BL:(m + 1) * BL, 1:, :] = o.transpose(2, 4, 1, 0, 3).reshape(
            BL, L - 1, 3)
    return y


# revision 7
# speedup vs baseline: 1.4249x; 1.0120x over previous
"""Holt-Winters exponential smoothing (level/trend/seasonal, P=7) on 8 Trainium2
NeuronCores — v11: v6 with bf16-hi-only group state (no lo plane).

Math identical to v3 (C=105 chunks, 13/group, 3 groups, channel-planar
pass-2, bf16 weights/x/out, fp32 PSUM, sigma_hi-only pass-2, bf16-hi-only
group state; validated relL2 4.8e-3 against float64, gate 2e-2).
Schedule changes vs v3:
  - all weights + s0 coalesced into ONE dram tensor / ONE dma (v3 lost
    35 us to a 13-dma weight-load stall before the first matmul);
  - input group 0 split across the sync and gpsimd queues so the first
    scan starts ~14 us in;
  - PSUM retiled to one-bank tiles: scan 2 x (126,512), pass-2 6 x
    (105,512) -- a 6-deep matmul->cast pipeline that decouples PE pacing
    from cast turnaround (HAM cold-clock trap);
  - output casts alternate DVE / ACT per tile.
"""

import numpy as np

P = 7
C = 105
G = 13
NG = 3
NCH = G * NG
KS = 114          # pass-2 rhs rows: 105 X + 9 sigma_hi
L = 4096
B = 8192
NCORES = 8
BL = B // NCORES
NH = 512

# wall (weights+s0) column offsets
WP0 = 0           # (114, 315)
WQ0 = 315         # (105, 1638)
WS0 = WP0 + 315 + 1638          # ws1 (9, 126)
S00 = WS0 + 126                 # s0 (9, 2048)
WALLW = S00 + 2 * BL            # 4127


def _sigmoid(z):
    return 1.0 / (1.0 + np.exp(-z))


def _step_mats(a, b, g):
    A, c = [], []
    for i in range(P):
        col = 2 + i
        Ai = np.zeros((9, 9), np.float64)
        ci = np.zeros(9, np.float64)
        Ai[0, 0] = 1 - a
        Ai[0, 1] = 1 - a
        Ai[0, col] += -a
        Ai[1, 0] = -a * b
        Ai[1, 1] = 1 - a * b
        Ai[1, col] += -a * b
        for j in range(P):
            Ai[2 + j, 2 + j] = 1.0
        Ai[col, :] = 0.0
        Ai[col, 0] = -g * (1 - a)
        Ai[col, 1] = -g * (1 - a)
        Ai[col, col] = g * a + 1 - g
        ci[0] = a
        ci[1] = a * b
        ci[col] = g * (1 - a)
        A.append(Ai)
        c.append(ci)
    return A, c


def _build_coeffs(alpha, beta, gamma):
    """Weight blocks in float64; packed into the per-core wall later."""
    a, b, g = _sigmoid(alpha), _sigmoid(beta), _sigmoid(gamma)
    A, c = _step_mats(a, b, g)
    slots = [(1 + k) % P for k in range(C)]

    Phi = np.zeros((C, 9, 9), np.float64)
    w = np.zeros((C, C, 9), np.float64)
    cur = np.eye(9)
    for k in range(C):
        i = slots[k]
        if k > 0:
            w[k, :k] = w[k - 1, :k] @ A[i].T
        w[k, k] = c[i]
        cur = A[i] @ cur
        Phi[k] = cur
    T = Phi[C - 1]
    V = w[C - 1].T.copy()

    wp = np.zeros((KS, 3 * C), np.float64)          # [ch0|ch1|ch2]
    for k in range(C):
        sel = [0, 1, 2 + slots[k]]
        for ch in range(3):
            wp[105:114, ch * C + k] = Phi[k][sel[ch]]
            for j in range(k + 1):
                wp[j, ch * C + k] = w[k, j][sel[ch]]

    Tpow = [np.eye(9)]
    for _ in range(G + 1):
        Tpow.append(T @ Tpow[-1])

    ws1 = np.zeros((9, 126), np.float64)
    ws1[:, 0:9] = Tpow[G].T
    for j in range(G):
        ws1[:, 9 + 9 * j:18 + 9 * j] = Tpow[j].T
    wq = np.zeros((C, G * 126), np.float64)         # [i0|i1|...|i12]
    for i in range(G):
        blk = wq[:, i * 126:(i + 1) * 126]
        blk[:, 0:9] = (Tpow[G - 1 - i] @ V).T
        for j in range(i + 1, G):
            blk[:, 9 + 9 * j:18 + 9 * j] = (Tpow[j - 1 - i] @ V).T

    return wp, wq, ws1


def build_bass(bl=BL):
    import concourse.bacc as bacc
    import concourse.mybir as mybir
    from concourse.tile import TileContext

    BF = mybir.dt.bfloat16
    F32 = mybir.dt.float32
    COPY = mybir.ActivationFunctionType.Copy
    nh = bl // NH
    GW = G * bl

    nc = bacc.Bacc(None, target_bir_lowering=False, debug=False)
    xin = nc.declare_dram_parameter("xin", [C, NCH * bl], BF, isOutput=False)
    wall_d = nc.declare_dram_parameter("wall", [KS, WALLW], BF,
                                       isOutput=False)
    out_d = nc.declare_dram_parameter("out", [C, NCH * 3 * bl], BF,
                                      isOutput=True)

    with TileContext(nc) as tc:
        with (
            tc.tile_pool(name="consts", bufs=1) as consts,
            tc.tile_pool(name="xpool", bufs=NG) as xpool,
            tc.tile_pool(name="spool", bufs=2) as spool,
            tc.tile_pool(name="ypool", bufs=4) as ypool,
            tc.tile_pool(name="ypsum", bufs=6, space="PSUM") as ypsum,
            tc.tile_pool(name="spsum", bufs=2, space="PSUM") as spsum,
        ):
            cw = consts.tile([KS, WALLW], BF)
            # s0 block first (tiny, gates the first scan matmuls), then the
            # weight block; rows 9:114 of the s0 region are never read, so
            # skip them instead of DMAing 430KB of zero padding.
            nc.sync.dma_start(out=cw[0:9, S00:WALLW], in_=wall_d[0:9, S00:WALLW])
            nc.sync.dma_start(out=cw[:, 0:S00], in_=wall_d[:, 0:S00])
            wp = cw[:, WP0:WP0 + 3 * C]
            wq = cw[0:C, WQ0:WQ0 + G * 126]
            ws1 = cw[0:9, WS0:WS0 + 126]
            s0 = cw[0:9, S00:S00 + 2 * bl]

            # fine-grained input loads: 2-chunk pieces for g0 (fast start,
            # 4KB partition lines), 3-4-chunk pieces for g1/g2; spread over
            # the sync and gpsimd queues to engage all engines early.
            xg = []
            pieces = {0: ([(0, 2, 1), (2, 4, 1), (4, 6, 1), (6, 8, 1),
                           (8, 10, 1), (10, 12, 1), (12, 13, 1)]),
                      1: ([(0, 4, 0), (4, 8, 0), (8, 11, 0), (11, 13, 0)]),
                      2: ([(0, 4, 1), (4, 8, 1), (8, 11, 1), (11, 13, 1)])}
            for g_ in range(NG):
                xt = xpool.tile([KS, GW], BF, tag="xg", name=f"xg{g_}")
                for (a, b, q) in pieces[g_]:
                    src = xin[:, g_ * GW + a * bl:g_ * GW + b * bl]
                    dst = xt[0:C, a * bl:b * bl]
                    if q == 0:
                        nc.sync.dma_start(out=dst, in_=src)
                    else:
                        nc.gpsimd.dma_start(out=dst, in_=src)
                xg.append(xt)

            state = [s0[:, 0:bl]]

            def scan_split_scatter(g_):
                st_h = state[g_]
                sps = []
                for h in range(nh):
                    hs = slice(h * NH, (h + 1) * NH)
                    sp = spsum.tile([126, NH], F32, tag="sp",
                                    name=f"sp{g_}_{h}")
                    nc.tensor.matmul(sp[:], lhsT=ws1[:], rhs=st_h[:, hs],
                                     start=True, stop=False)
                    for i in range(G):
                        nc.tensor.matmul(sp[:],
                                         lhsT=wq[:, i * 126:(i + 1) * 126],
                                         rhs=xg[g_][0:C, i * bl + h * NH:
                                                    i * bl + h * NH + NH],
                                         start=False, stop=(i == G - 1))
                    sps.append(sp)
                sg = spool.tile([126, bl], BF, tag="sg", name=f"sg{g_}")
                for h in range(nh):
                    hs = slice(h * NH, (h + 1) * NH)
                    nc.scalar.activation(out=sg[:, hs], in_=sps[h][:],
                                         func=COPY)
                for i in range(G):
                    nc.sync.dma_start(
                        out=xg[g_][105:114, i * bl:(i + 1) * bl],
                        in_=sg[9 + 9 * i:18 + 9 * i, :])
                state.append(sg[0:9, :])

            def pass2_chunk(g_, i):
                ot = ypool.tile([C, 3 * bl], BF, tag="ot", name=f"ot{g_}_{i}")
                k = 0
                for h in range(nh):
                    chs = (0, 1, 2) if h == 0 else (2, 1, 0)
                    for ch in chs:
                        yp = ypsum.tile([C, NH], F32, tag="yp",
                                        name=f"yp{g_}_{i}_{h}_{ch}")
                        nc.tensor.matmul(
                            yp[:],
                            lhsT=wp[:, ch * C:(ch + 1) * C],
                            rhs=xg[g_][0:KS, i * bl + h * NH:
                                       i * bl + h * NH + NH],
                            start=True, stop=True)
                        oc = slice((h * 3 + ch) * NH, (h * 3 + ch + 1) * NH)
                        if k % 2 == 0:
                            nc.vector.tensor_copy(out=ot[:, oc], in_=yp[:])
                        else:
                            nc.scalar.activation(out=ot[:, oc], in_=yp[:],
                                                 func=COPY)
                        k += 1
                c0 = (g_ * G + i) * 3 * bl
                if g_ == NG - 1 and i % 2 == 1:
                    nc.sync.dma_start(out=out_d[:, c0:c0 + 3 * bl], in_=ot[:])
                else:
                    nc.gpsimd.dma_start(out=out_d[:, c0:c0 + 3 * bl],
                                        in_=ot[:])

            scan_split_scatter(0)
            for g_ in range(NG):
                for i in range(G):
                    if i == 3 and g_ + 1 < NG:
                        scan_split_scatter(g_ + 1)
                    pass2_chunk(g_, i)
    nc.compile()
    return nc


def _prep_inputs(x, alpha, beta, gamma):
    import ml_dtypes
    bf = ml_dtypes.bfloat16
    xs = np.asarray(x, dtype=np.float32).reshape(B, L)
    wp, wq, ws1 = _build_coeffs(float(alpha), float(beta), float(gamma))
    wall0 = np.zeros((KS, WALLW), np.float32)
    wall0[:, WP0:WP0 + 3 * C] = wp
    wall0[0:C, WQ0:WQ0 + G * 126] = wq
    wall0[0:9, WS0:WS0 + 126] = ws1
    in_maps = []
    for m in range(NCORES):
        xm = xs[m * BL:(m + 1) * BL]
        xT = np.ascontiguousarray(xm.T)
        xb = xT.astype(bf)
        xin = np.ascontiguousarray(
            xb[1:L].reshape(NCH, C, BL).transpose(1, 0, 2)).reshape(
                C, NCH * BL)
        s0 = np.zeros((9, BL), np.float32)
        s0[0] = xT[0]
        s0[1] = xT[1] - xT[0]
        for j in range(1, P):
            s0[2 + j] = xT[j] - xT[0]
        s0h = s0.astype(bf)
        s0l = (s0 - s0h.astype(np.float32)).astype(bf)
        wall = wall0.copy()
        wall[0:9, S00:S00 + BL] = s0h
        wall[0:9, S00 + BL:S00 + 2 * BL] = s0l
        in_maps.append({"xin": xin, "wall": wall.astype(bf)})
    return in_maps


LAST_RESULT = None

def _ensure_ntff_hook():
    """If BASS_TRACE is set but this environment lacks antenv.axon_hooks
    (concourse imports it under axon when tracing), provide it -- registered
    from the injected libaxon_pjrt.so when available, else a no-op so
    run_bass_kernel_spmd degrades to an untraced run instead of crashing."""
    import importlib.util
    try:
        if importlib.util.find_spec("antenv.axon_hooks") is not None:
            return
    except (ImportError, ModuleNotFoundError, ValueError):
        pass
    import contextlib
    import ctypes
    import sys
    import types

    mod = types.ModuleType("antenv.axon_hooks")
    mod._hook = None
    mod.set_axon_ntff_profile_hook = lambda h: setattr(mod, "_hook", h)
    mod.get_axon_ntff_profile_hook = lambda: mod._hook
    sys.modules["antenv.axon_hooks"] = mod
    try:
        import antenv
        antenv.axon_hooks = mod
    except ImportError:
        pass
    try:
        lib = ctypes.CDLL("/opt/axon/libaxon_pjrt.so")
        if not hasattr(lib, "axon_start_nrt_profile"):
            return
        lib.axon_start_nrt_profile.argtypes = [
            ctypes.POINTER(ctypes.c_int64), ctypes.c_size_t]
        lib.axon_start_nrt_profile.restype = ctypes.c_int64
        lib.axon_stop_nrt_profile.argtypes = [ctypes.c_char_p]
        lib.axon_stop_nrt_profile.restype = ctypes.c_int64

        @contextlib.contextmanager
        def _hook(output_dir, device_ids):
            import jax
            jax.devices()
            if device_ids:
                ids = (ctypes.c_int64 * len(device_ids))(*device_ids)
                rc = lib.axon_start_nrt_profile(ids, len(device_ids))
            else:
                rc = lib.axon_start_nrt_profile(None, 0)
            if rc != 0:
                raise RuntimeError(f"axon_start_nrt_profile rc={rc}")
            try:
                yield
            finally:
                lib.axon_stop_nrt_profile(str(output_dir).encode())

        mod.set_axon_ntff_profile_hook(_hook)
    except OSError:
        pass



def kernel(x, alpha, beta, gamma):
    global LAST_RESULT
    _ensure_ntff_hook()
    from concourse.bass_utils import run_bass_kernel_spmd

    nc = build_bass(BL)
    in_maps = _prep_inputs(x, alpha, beta, gamma)
    res = run_bass_kernel_spmd(nc, in_maps, core_ids=list(range(NCORES)))
    LAST_RESULT = res
    xs = np.asarray(x, dtype=np.float32).reshape(B, L)
    y = np.empty((B, L, 3), np.float32)
    y[:, 0, 0] = xs[:, 0]
    y[:, 0, 1] = xs[:, 1] - xs[:, 0]
    y[:, 0, 2] = 0.0
    for m in range(NCORES):
        o = res.results[m]["out"]
        o = o.reshape(C, NCH, 2, 3, NH).astype(np.float32)
        y[m * BL:(m + 1) * BL, 1:, :] = o.transpose(2, 4, 1, 0, 3).reshape(
            BL, L - 1, 3)
    return y
